# revision 5
# baseline (speedup 1.0000x reference)
"""Stereo cost volume on 8 Trainium2 NeuronCores (batch-parallel SPMD).

out[b,h,w,d] = sum_c ref[b,h,w+63-d,c] * aux[b,h,w,c]
  B=8, H=192, W=384, C=128, D=64, ref width 447.

Strategy:
  * Shard batch across the 8 cores (1 batch each); pure SPMD, no collectives.
  * Host pre-transposes inputs to [C, H, W] fp16 so the channel contraction
    (C=128) lands on SBUF partitions and feeds the 128x128 PE array exactly.
  * Per h-row, per 128-wide W chunk: 4 col-tiled matmuls (M=32 output
    positions each, tile_position=(0,32g)) stream a 95-column ref window into
    one PSUM bank laid out [128, 288].  Grouping output w-positions by 32
    bounds each group's diagonal band inside 95 uniform columns, so no
    per-partition (diagonal) addressing is ever needed on device.
  * One DVE copy PSUM->SBUF per h, large contiguous DMAs in/out.
  * Host extracts the diagonal band from the [128, H, 288] per-core output
    with a zero-copy as_strided view (shear is free on the host).
"""

import sys

import numpy as np

sys.path.insert(0, "/opt/trn_rl_repo")

import concourse.bass as bass
import concourse.mybir as mybir
from concourse import bacc, bass_utils
from concourse.tile import TileContext

B, H, W, C, D = 8, 192, 384, 128, 64
OFF = 63
REF_W = W + OFF  # 447
NCHUNK = W // 128  # 3
GW = 32  # output w-positions per col group
NGROUP = 128 // GW  # 4
WIN = GW + OFF  # 95 streamed ref columns per group
BLK = 96  # column stride per chunk block in the shipped tile
OUT_COLS = NCHUNK * BLK  # 288
HB = 16  # h rows per DMA block

F16 = mybir.dt.float16
F32 = mybir.dt.float32


def _build() -> bass.Bass:
    nc = bacc.Bacc("TRN2", target_bir_lowering=False, debug=False)
    ref_d = nc.dram_tensor("ref_t", [C, H, REF_W], F16, kind="ExternalInput").ap()
    aux_d = nc.dram_tensor("aux_t", [C, H, W], F16, kind="ExternalInput").ap()
    out_d = nc.dram_tensor("out_raw", [128, H, OUT_COLS], F32, kind="ExternalOutput").ap()

    with TileContext(nc) as tc:
        with (
            tc.tile_pool(name="inp", bufs=2) as inp,
            tc.tile_pool(name="outp", bufs=2) as outp,
            tc.tile_pool(name="ps", bufs=4, space="PSUM") as ps,
        ):
            for hb in range(0, H, HB):
                ref_sb = inp.tile([C, HB * REF_W], F16, tag="ref")
                aux_sb = inp.tile([C, HB * W], F16, tag="aux")
                nc.sync.dma_start(out=ref_sb, in_=ref_d[:, hb : hb + HB, :])
                nc.sync.dma_start(out=aux_sb, in_=aux_d[:, hb : hb + HB, :])
                out_sb = outp.tile([128, HB * OUT_COLS], F32, tag="out")
                for hl in range(HB):
                    pt = ps.tile([128, OUT_COLS], F32)
                    for k in range(NCHUNK):
                        for g in range(NGROUP):
                            w0 = 128 * k + GW * g
                            nc.tensor.matmul(
                                out=pt[GW * g : GW * g + GW, BLK * k : BLK * k + WIN],
                                lhsT=aux_sb[:, hl * W + w0 : hl * W + w0 + GW],
                                rhs=ref_sb[:, hl * REF_W + w0 : hl * REF_W + w0 + WIN],
                                start=True,
                                stop=True,
                                tile_position=(0, GW * g),
                            )
                    nc.vector.tensor_copy(
                        out=out_sb[:, hl * OUT_COLS : (hl + 1) * OUT_COLS], in_=pt
                    )
                nc.sync.dma_start(out=out_d[:, hb : hb + HB, :], in_=out_sb)
    nc.compile()
    return nc


def _extract(core_out: np.ndarray) -> np.ndarray:
    """[128, H, 288] f32 device output -> [H, W, D] cost volume (one batch).

    Device row m = 32g + r, column 96k + c holds
    dot(aux[128k + 32g + r], ref[128k + 32g + c]); the band entry for
    w = 128k + 32g + r, disparity d sits at c = r + 63 - d.
    """
    sm, sh, sc = core_out.strides
    base = core_out[:, :, OFF:]
    v = np.lib.stride_tricks.as_strided(
        base,
        shape=(H, NCHUNK, NGROUP, GW, D),
        strides=(sh, BLK * sc, GW * sm, sm + sc, -sc),
    )
    return v.reshape(H, W, D)


LAST_RESULTS = None


def kernel(ref: np.ndarray, aux: np.ndarray, _trace: bool = False) -> np.ndarray:
    global LAST_RESULTS
    ref16 = np.ascontiguousarray(ref.astype(np.float16).transpose(0, 3, 1, 2))
    aux16 = np.ascontiguousarray(aux.astype(np.float16).transpose(0, 3, 1, 2))
    nc = _build()
    in_maps = [{"ref_t": ref16[b], "aux_t": aux16[b]} for b in range(B)]
    res = bass_utils.run_bass_kernel_spmd(nc, in_maps, list(range(B)), trace=_trace)
    LAST_RESULTS = res
    return np.stack([_extract(res.results[b]["out_raw"]) for b in range(B)], axis=0)


# revision 8
# speedup vs baseline: 1.0723x; 1.0723x over previous
"""Stereo cost volume on 8 Trainium2 NeuronCores (batch-parallel SPMD).

out[b,h,w,d] = sum_c ref[b,h,w+63-d,c] * aux[b,h,w,c]
  B=8, H=192, W=384, C=128, D=64, ref width 447.

Strategy:
  * Shard batch across the 8 cores (1 batch each); pure SPMD, no collectives.
  * Host pre-transposes inputs to [C, H, W] fp16 so the channel contraction
    (C=128) lands on SBUF partitions and feeds the 128x128 PE array exactly.
  * Per h-row, per 128-wide W chunk: 4 col-tiled matmuls (M=32 output
    positions each, tile_position=(0,32g)) stream a 95-column ref window into
    one PSUM bank laid out [128, 288].  Grouping output w-positions by 32
    bounds each group's diagonal band inside 95 uniform columns, so no
    per-partition (diagonal) addressing is ever needed on device.
  * One DVE copy PSUM->SBUF per h, large contiguous DMAs in/out.
  * Host extracts the diagonal band from the [128, H, 288] per-core output
    with a zero-copy as_strided view (shear is free on the host).
"""

import sys

import numpy as np

sys.path.insert(0, "/opt/trn_rl_repo")

import concourse.bass as bass
import concourse.mybir as mybir
from concourse import bacc, bass_utils
from concourse.tile import TileContext

B, H, W, C, D = 8, 192, 384, 128, 64
OFF = 63
REF_W = W + OFF  # 447
NCHUNK = W // 128  # 3
GW = 32  # output w-positions per col group
NGROUP = 128 // GW  # 4
WIN = GW + OFF  # 95 streamed ref columns per group
BLK = 96  # column stride per chunk block in the shipped tile
OUT_COLS = NCHUNK * BLK  # 288
HB = 8  # h rows per DMA block

F16 = mybir.dt.float16
F32 = mybir.dt.float32


def _build() -> bass.Bass:
    nc = bacc.Bacc("TRN2", target_bir_lowering=False, debug=False)
    ref_d = nc.dram_tensor("ref_t", [C, H, REF_W], F16, kind="ExternalInput").ap()
    aux_d = nc.dram_tensor("aux_t", [C, H, W], F16, kind="ExternalInput").ap()
    out_d = nc.dram_tensor("out_raw", [128, H, OUT_COLS], F32, kind="ExternalOutput").ap()

    with TileContext(nc) as tc:
        with (
            tc.tile_pool(name="inp", bufs=3) as inp,
            tc.tile_pool(name="outp", bufs=3) as outp,
            tc.tile_pool(name="ps", bufs=6, space="PSUM") as ps,
        ):
            for hb in range(0, H, HB):
                ref_sb = inp.tile([C, HB * REF_W], F16, tag="ref")
                aux_sb = inp.tile([C, HB * W], F16, tag="aux")
                nc.sync.dma_start(out=ref_sb, in_=ref_d[:, hb : hb + HB, :])
                nc.sync.dma_start(out=aux_sb, in_=aux_d[:, hb : hb + HB, :])
                out_sb = outp.tile([128, HB * OUT_COLS], F32, tag="out")
                for hl in range(HB):
                    pt = ps.tile([128, OUT_COLS], F32)
                    for k in range(NCHUNK):
                        for g in range(NGROUP):
                            w0 = 128 * k + GW * g
                            nc.tensor.matmul(
                                out=pt[GW * g : GW * g + GW, BLK * k : BLK * k + WIN],
                                lhsT=aux_sb[:, hl * W + w0 : hl * W + w0 + GW],
                                rhs=ref_sb[:, hl * REF_W + w0 : hl * REF_W + w0 + WIN],
                                start=True,
                                stop=True,
                                tile_position=(0, GW * g),
                            )
                    nc.vector.tensor_copy(
                        out=out_sb[:, hl * OUT_COLS : (hl + 1) * OUT_COLS], in_=pt
                    )
                # outputs go out on the Activation HWDGE queue so they don't
                # serialize behind input loads on the sync queue
                nc.scalar.dma_start(out=out_d[:, hb : hb + HB, :], in_=out_sb)
    nc.compile()
    return nc


def _extract(core_out: np.ndarray) -> np.ndarray:
    """[128, H, 288] f32 device output -> [H, W, D] cost volume (one batch).

    Device row m = 32g + r, column 96k + c holds
    dot(aux[128k + 32g + r], ref[128k + 32g + c]); the band entry for
    w = 128k + 32g + r, disparity d sits at c = r + 63 - d.
    """
    sm, sh, sc = core_out.strides
    base = core_out[:, :, OFF:]
    v = np.lib.stride_tricks.as_strided(
        base,
        shape=(H, NCHUNK, NGROUP, GW, D),
        strides=(sh, BLK * sc, GW * sm, sm + sc, -sc),
    )
    return v.reshape(H, W, D)


LAST_RESULTS = None


def kernel(ref: np.ndarray, aux: np.ndarray, _trace: bool = False) -> np.ndarray:
    global LAST_RESULTS
    ref16 = np.ascontiguousarray(ref.astype(np.float16).transpose(0, 3, 1, 2))
    aux16 = np.ascontiguousarray(aux.astype(np.float16).transpose(0, 3, 1, 2))
    nc = _build()
    in_maps = [{"ref_t": ref16[b], "aux_t": aux16[b]} for b in range(B)]
    res = bass_utils.run_bass_kernel_spmd(nc, in_maps, list(range(B)), trace=_trace)
    LAST_RESULTS = res
    return np.stack([_extract(res.results[b]["out_raw"]) for b in range(B)], axis=0)
